# revision 19
# baseline (speedup 1.0000x reference)
"""Trainium2 Bass kernel for nn_DMS_STGAT (dual-branch GAT attention softmaxes).

Strategy (per core, data-parallel over batch B=16 -> 2 per core):
  The reference only uses h = x @ W through two dots s1 = h@a[:F], s2 = h@a[F:],
  so  e[bt, n1, n2] = LRelu(s1[r1[m]] + s2[r2[m]])  with fixed index maps r1/r2
  (the "scrambled pairing").  We compute s1/s2 as 128-dim dots with
  host-precomputed Wa = W@a vectors, gather via tiny host-precomputed 0/1
  matrices Q1/Q2 using stacked-K PE matmuls, then run the double softmax
  on-chip.  Spatial PE term exp(-||x_j - x_8||/1000) enters as extra stacked
  rows with Qs = S1*Q1 + S2*Q2.  Temporal positional constant qp enters as a
  ones-row; overflow safety comes from a post-LRelu per-group constant shift.
"""
import sys
import numpy as np

for _p in ("/opt/trn_rl_repo", "/root/.axon_site/_ro/trn_rl_repo"):
    if _p not in sys.path:
        sys.path.insert(0, _p)

from contextlib import ExitStack  # noqa: E402

import concourse.bass as bass  # noqa: E402
import concourse.tile as tile  # noqa: E402
from concourse import bacc, mybir  # noqa: E402

B, C, T, J, F = 16, 128, 25, 25, 256
N = 25            # N == T == J
NN = N * N        # 625
NCORES = 8
BL = B // NCORES  # 2 batches per core
FP = mybir.dt.float32
BF = mybir.dt.bfloat16
AF = mybir.ActivationFunctionType
ALU = mybir.AluOpType

# Stacked-K layouts for the two E matmuls
KS = 89           # spatial: rows 0:25 s1, 25:50 s2, 50:64 zero, 64:89 ec
KT = 57           # temporal: rows 0:25 t1, 25 ones, 26:32 zero, 32:57 t2

# Pin ALL activation functions to one table set (exp/ln/square/copy live
# together in natural_log_exp_and_others) so only one ACT_TABLE_LOAD happens.
_orig_get_tables = bacc.get_activation_tables


def _pinned_tables(arch):
    tabs = dict(_orig_get_tables(arch))
    assert "natural_log_exp_and_others" in tabs
    return {k: (v if k == "natural_log_exp_and_others" else set())
            for k, v in tabs.items()}


bacc.get_activation_tables = _pinned_tables

# ---------------------------------------------------------------- host math --

def _pair_indices():
    """r1[m], r2[m] for m = n1*N + n2 (original flat order)."""
    r1 = np.zeros(NN, np.int64)
    r2 = np.zeros(NN, np.int64)
    for m in range(NN):
        k1, k2 = 2 * m, 2 * m + 1
        r1[m] = (k1 // N) if k1 < NN else ((k1 - NN) % N)
        r2[m] = (k2 // N) if k2 < NN else ((k2 - NN) % N)
    return r1, r2


def _sinusoid_pos():
    pos = np.arange(200)[:, None].astype(np.float64)
    hid = np.arange(C)[None, :]
    angle = pos / np.power(10000.0, 2.0 * (hid // 2) / C)
    tab = angle.copy()
    tab[:, 0::2] = np.sin(angle[:, 0::2])
    tab[:, 1::2] = np.cos(angle[:, 1::2])
    return tab[:T] * 1000.0  # [T, C] float64


_R1, _R2 = _pair_indices()


def _host_consts(W_s, a_s, W_t, a_t):
    """Precompute tiny derived params in float64. ~0.3 MFLOP."""
    W_s = W_s.astype(np.float64)
    a_s = a_s.astype(np.float64)
    W_t = W_t.astype(np.float64)
    a_t = a_t.astype(np.float64)
    wa_s1 = W_s @ a_s[:F, 0]
    wa_s2 = W_s @ a_s[F:, 0]
    wa_t1 = W_t @ a_t[:F, 0]
    wa_t2 = W_t @ a_t[F:, 0]
    S1, S2 = wa_s1.sum(), wa_s2.sum()

    Q1 = np.zeros((N, NN), np.float64)
    Q2 = np.zeros((N, NN), np.float64)
    Q1[_R1, np.arange(NN)] = 1.0
    Q2[_R2, np.arange(NN)] = 1.0
    qs = S1 * Q1 + S2 * Q2                      # [25, 625]

    pos = _sinusoid_pos()                       # [25, 128]
    p1 = pos @ wa_t1
    p2 = pos @ wa_t2
    qp = p1[_R1] + p2[_R2]                      # [625] true temporal constant
    # post-LRelu shift constant, per n2-group (softmax-invariant, exp-safe)
    qLR = np.where(qp > 0, qp, 0.2 * qp)
    cq = qLR.reshape(N, N).max(axis=0)          # max over n1, per n2
    csh = cq[np.arange(NN) % N][None, :]        # [1, 625]

    wa4 = np.stack([wa_s1, wa_s2, wa_t1, wa_t2], axis=1)  # [128, 4]
    # stacked rhs matrices
    qstk_s = np.zeros((KS, NN), np.float64)
    qstk_s[0:N] = Q1
    qstk_s[N:2 * N] = Q2
    qstk_s[64:64 + N] = qs
    qstk_t = np.zeros((KT, NN), np.float64)
    qstk_t[0:N] = Q1
    qstk_t[N] = qp
    qstk_t[32:32 + N] = Q2
    return (wa4.astype(np.float32), qstk_s.astype(np.float32),
            qstk_t.astype(np.float32), csh.astype(np.float32))


# ------------------------------------------------------------- bass program --

def _build_program():
    nc = bacc.Bacc("TRN2", target_bir_lowering=False, debug=False)

    src_d = nc.dram_tensor("src_l", [BL, C, T, J], FP, kind="ExternalInput").ap()
    wa4_d = nc.dram_tensor("wa4", [C, 4], FP, kind="ExternalInput").ap()
    qss_d = nc.dram_tensor("qstk_s", [KS, NN], FP, kind="ExternalInput").ap()
    qst_d = nc.dram_tensor("qstk_t", [KT, NN], FP, kind="ExternalInput").ap()
    csh_d = nc.dram_tensor("csh", [1, NN], FP, kind="ExternalInput").ap()
    outs_d = nc.dram_tensor("out_s", [BL, T, N, N], FP, kind="ExternalOutput").ap()
    outt_d = nc.dram_tensor("out_t", [BL, T, N, N], FP, kind="ExternalOutput").ap()

    with tile.TileContext(nc) as tc, ExitStack() as ctx:
        consts = ctx.enter_context(tc.tile_pool(name="consts", bufs=1))
        data = ctx.enter_context(tc.tile_pool(name="data", bufs=1))
        pp = ctx.enter_context(tc.tile_pool(name="pp", bufs=1, space="PSUM"))

        # --- constants / warm-up ---
        dummy = consts.tile([1, 2], FP)
        nc.vector.memset(dummy[:], 0.0)
        nc.scalar.activation(dummy[:], dummy[:], AF.Exp)  # pull ACT table load early

        wa4 = consts.tile([C, 4], FP)
        nc.sync.dma_start(wa4[:], wa4_d)
        qss = consts.tile([KS, NN], FP)
        nc.sync.dma_start(qss[:], qss_d)
        qst = consts.tile([KT, NN], FP)
        nc.sync.dma_start(qst[:], qst_d)
        ones_bf = consts.tile([C, 1], BF)
        nc.vector.memset(ones_bf[:], 1.0)
        # temporal post-LRelu shift constant, broadcast to partitions 64:114
        CSHt = consts.tile([114, NN], FP)
        csh_b = bass.AP(tensor=csh_d.tensor, offset=csh_d.offset, ap=[[0, 50], [1, NN]])
        nc.gpsimd.dma_start(CSHt[64:114, :], csh_b)

        # --- input: X [128, (b, t, j)] ---
        X = data.tile([C, BL * NN], FP)
        for b in range(BL):
            src_b = bass.AP(tensor=src_d.tensor, offset=src_d.offset + b * C * NN,
                            ap=[[NN, C], [1, NN]])
            nc.sync.dma_start(X[:, b * NN:(b + 1) * NN], src_b)
        FX = X[:].ap[0][0]

        # --- X_jt [128, (b, j, t)] for the temporal pass (ACT strided copy) ---
        X_jt = data.tile([C, BL * NN], FP)
        xin = bass.AP(tensor=X.tensor, offset=X.offset,
                      ap=[[FX, C], [NN, BL], [1, N], [N, N]])   # (c, b, j, t)
        nc.scalar.copy(X_jt[:], xin)

        # --- D2 = (X - ref)^2 in bf16 (spatial PE distance), per b ---
        D = data.tile([C, BL * NN], FP)
        D2 = data.tile([C, BL * NN], BF)
        FD = D[:].ap[0][0]
        for b in range(BL):
            in0 = bass.AP(tensor=X.tensor, offset=X.offset + b * NN,
                          ap=[[FX, C], [N, N], [1, N]])
            ref = bass.AP(tensor=X.tensor, offset=X.offset + b * NN + 8,
                          ap=[[FX, C], [N, N], [0, N]])
            dout = bass.AP(tensor=D.tensor, offset=D.offset + b * NN,
                           ap=[[FD, C], [N, N], [1, N]])
            nc.gpsimd.tensor_tensor(dout, in0, ref, op=ALU.subtract)
            nc.vector.tensor_tensor(D2[:, b * NN:(b + 1) * NN],
                                    D[:, b * NN:(b + 1) * NN],
                                    D[:, b * NN:(b + 1) * NN], op=ALU.mult)

        # --- dot-product passes (PE), chunked stationary ---
        # E-psum allocated first so its 512-col chunks are bank-aligned
        psum_E = pp.tile([114, 1024], FP)  # 2 full banks; cols 0:625 used
        # spatial pass: chunks of 125 cols; psum_sd [125, 30]:
        #   col (b*5+ck)*3 + {0:s1, 1:s2, 2:d2}
        psum_sd = pp.tile([125, 30], FP)
        for q in range(BL * 5):
            nc.tensor.matmul(psum_sd[:, q * 3:q * 3 + 2],
                             X[:, q * 125:(q + 1) * 125], wa4[:, 0:2],
                             start=True, stop=True)
        # temporal pass on X_jt: psum_td [125 (js,t), 20]: col (b*5+jq)*2 + d
        psum_td = pp.tile([125, 20], FP)
        for q in range(BL * 5):
            nc.tensor.matmul(psum_td[:, q * 2:q * 2 + 2],
                             X_jt[:, q * 125:(q + 1) * 125], wa4[:, 2:4],
                             start=True, stop=True)
        # d2 pass (bf16, single-pass)
        for q in range(BL * 5):
            nc.tensor.matmul(psum_sd[:, q * 3 + 2:q * 3 + 3],
                             D2[:, q * 125:(q + 1) * 125], ones_bf[:],
                             start=True, stop=True)

        # --- PSUM -> SBUF ---
        SD = data.tile([125, 30], FP)
        nc.scalar.copy(SD[:], psum_sd[:])
        TD = data.tile([125, 20], FP)
        nc.vector.tensor_copy(TD[:], psum_td[:])

        # --- stacked lhsT tiles (zero-filled, then rearranged in) ---
        SPK = data.tile([KS, 50], FP)     # rows: s1 / s2 / 0 / ec(from d2s)
        nc.vector.memset(SPK[:], 0.0)
        TPK = data.tile([KT, 50], FP)     # rows: t1 / ones / 0 / t2
        nc.vector.memset(TPK[:], 0.0)
        onesrow = consts.tile([1, 50], FP)
        nc.vector.memset(onesrow[:], 1.0)
        nc.sync.dma_start(TPK[N:N + 1, :], onesrow[:])

        FSD = SD[:].ap[0][0]
        FTD = TD[:].ap[0][0]
        FSK = SPK[:].ap[0][0]
        FTK = TPK[:].ap[0][0]
        # spatial: SPK[row_d + j, (b*25+t)] = SD[ts*25+j, (b*5+ck)*3+d], t=ck*5+ts
        for ts in range(5):
            for d, rbase in ((0, 0), (1, N), (2, 64)):
                src = bass.AP(tensor=SD.tensor,
                              offset=SD.offset + (ts * 25) * FSD + d,
                              ap=[[FSD, N], [3, 10], [1, 1]])      # (j, bck)
                dst = bass.AP(tensor=SPK.tensor,
                              offset=SPK.offset + rbase * FSK + ts,
                              ap=[[FSK, N], [5, 10], [1, 1]])      # (j, bck)
                nc.sync.dma_start(dst, src)
        # temporal: TPK[row_d + t, b*25 + jq*5 + js] = TD[js*25+t, (b*5+jq)*2+d]
        for js in range(5):
            for d, rbase in ((0, 0), (1, 32)):
                src = bass.AP(tensor=TD.tensor,
                              offset=TD.offset + (js * 25) * FTD + d,
                              ap=[[FTD, N], [2, 10], [1, 1]])      # (t, bjq)
                dst = bass.AP(tensor=TPK.tensor,
                              offset=TPK.offset + rbase * FTK + js,
                              ap=[[FTK, N], [5, 10], [1, 1]])      # (t, bjq)
                nc.sync.dma_start(dst, src)

        # --- EC = exp(-sqrt(d2s)/1000) via exp(0.5*ln) on SPK rows 64:89 ---
        eps_b = consts.tile([89, 1], FP)
        nc.vector.memset(eps_b[:], 1e-30)
        ecL = data.tile([89, 50], FP)
        nc.scalar.activation(ecL[64:89, :], SPK[64:89, 0:50], AF.Ln,
                             bias=eps_b[64:89])
        ecW = data.tile([89, 50], FP)
        nc.scalar.activation(ecW[64:89, :], ecL[64:89, :], AF.Exp, scale=0.5)
        nc.scalar.activation(SPK[64:89, 0:50], ecW[64:89, :], AF.Exp, scale=-0.001)

        # --- E matmuls (stacked-K): spatial rows 0:50, temporal 64:114 ---
        nc.vector.memset(psum_E[32:64, 0:NN], 0.0)  # junk rows 50:64
        chunks = [(0, 512), (512, NN)]
        for lo, hi in chunks:
            nc.tensor.matmul(psum_E[0:50, lo:hi], SPK[:, :], qss[:, lo:hi],
                             start=True, stop=True)
        for lo, hi in chunks:
            nc.tensor.matmul(psum_E[64:114, lo:hi], TPK[:, :], qst[:, lo:hi],
                             start=True, stop=True, tile_position=(0, 64))

        # --- LRelu(0.2): E2 = max(E, 0.2*E); temporal rows get -csh ---
        t0 = data.tile([114, NN], FP)
        nc.scalar.mul(t0[:], psum_E[:, 0:NN], 0.2)
        E2 = data.tile([114, NN], FP)
        nc.vector.tensor_tensor(E2[:], psum_E[:, 0:NN], t0[:], op=ALU.max)
        nc.vector.tensor_tensor(E2[64:114, :], E2[64:114, :], CSHt[64:114, :],
                                op=ALU.subtract)

        # --- double softmax over n1 (stride-25 axis) ---
        g = data.tile([114, NN], FP)
        nc.scalar.activation(g[:], E2[:], AF.Exp)
        FG = g[:].ap[0][0]
        Z = data.tile([114, N], FP)
        g_red = bass.AP(tensor=g.tensor, offset=g.offset, ap=[[FG, 114], [1, N], [N, N]])
        nc.vector.tensor_reduce(Z[:], g_red, axis=mybir.AxisListType.X, op=ALU.add)
        Zr = data.tile([114, N], FP)
        nc.vector.reciprocal(Zr[:], Z[:])
        att1 = data.tile([114, NN], FP)
        FZ = Zr[:].ap[0][0]
        FA = att1[:].ap[0][0]
        g_3d = bass.AP(tensor=g.tensor, offset=g.offset, ap=[[FG, 114], [N, N], [1, N]])
        zr_b = bass.AP(tensor=Zr.tensor, offset=Zr.offset, ap=[[FZ, 114], [0, N], [1, N]])
        a1_3d = bass.AP(tensor=att1.tensor, offset=att1.offset, ap=[[FA, 114], [N, N], [1, N]])
        nc.vector.tensor_tensor(a1_3d, g_3d, zr_b, op=ALU.mult)

        g2 = data.tile([114, NN], FP)
        nc.scalar.activation(g2[:], att1[:], AF.Exp)
        FG2 = g2[:].ap[0][0]
        Z2 = data.tile([114, N], FP)
        g2_red = bass.AP(tensor=g2.tensor, offset=g2.offset, ap=[[FG2, 114], [1, N], [N, N]])
        nc.vector.tensor_reduce(Z2[:], g2_red, axis=mybir.AxisListType.X, op=ALU.add)
        Z2r = data.tile([114, N], FP)
        nc.vector.reciprocal(Z2r[:], Z2[:])
        outF = data.tile([114, NN], FP)
        FZ2 = Z2r[:].ap[0][0]
        FO = outF[:].ap[0][0]
        g2_3d = bass.AP(tensor=g2.tensor, offset=g2.offset, ap=[[FG2, 114], [N, N], [1, N]])
        z2_b = bass.AP(tensor=Z2r.tensor, offset=Z2r.offset, ap=[[FZ2, 114], [0, N], [1, N]])
        o_3d = bass.AP(tensor=outF.tensor, offset=outF.offset, ap=[[FO, 114], [N, N], [1, N]])
        nc.vector.tensor_tensor(o_3d, g2_3d, z2_b, op=ALU.mult)

        # --- outputs ---
        outs_flat = bass.AP(tensor=outs_d.tensor, offset=outs_d.offset,
                            ap=[[NN, 50], [1, NN]])
        outt_flat = bass.AP(tensor=outt_d.tensor, offset=outt_d.offset,
                            ap=[[NN, 50], [1, NN]])
        nc.sync.dma_start(outs_flat, outF[0:50, :])
        nc.sync.dma_start(outt_flat, outF[64:114, :])

    nc.compile()
    return nc


_PROGRAM = None


def _get_program():
    global _PROGRAM
    if _PROGRAM is None:
        _PROGRAM = _build_program()
    return _PROGRAM


# ------------------------------------------------------------------ kernel --

def kernel(src, W_s, a_s, W_t, a_t):
    from concourse.bass_utils import run_bass_kernel_spmd

    src = np.ascontiguousarray(np.asarray(src, dtype=np.float32))
    wa4, qstk_s, qstk_t, csh = _host_consts(np.asarray(W_s), np.asarray(a_s),
                                            np.asarray(W_t), np.asarray(a_t))
    nc = _get_program()
    in_maps = []
    for c in range(NCORES):
        in_maps.append({
            "src_l": src[c * BL:(c + 1) * BL],
            "wa4": wa4, "qstk_s": qstk_s, "qstk_t": qstk_t, "csh": csh,
        })
    res = run_bass_kernel_spmd(nc, in_maps, core_ids=list(range(NCORES)))
    out_s = np.concatenate([res.results[c]["out_s"] for c in range(NCORES)], axis=0)
    out_t = np.concatenate([res.results[c]["out_t"] for c in range(NCORES)], axis=0)
    return out_s, out_t


# revision 21
# speedup vs baseline: 1.0568x; 1.0568x over previous
"""Trainium2 Bass kernel for nn_DMS_STGAT (dual-branch GAT attention softmaxes).

Strategy (per core, data-parallel over batch B=16 -> 2 per core):
  The reference only uses h = x @ W through two dots s1 = h@a[:F], s2 = h@a[F:],
  so  e[bt, n1, n2] = LRelu(s1[r1[m]] + s2[r2[m]])  with fixed index maps r1/r2
  (the "scrambled pairing").  s1/s2 are 128-dim dots against host-precomputed
  Wa = W@a vectors, gathered via tiny host-precomputed 0/1 matrices using
  stacked-K PE matmuls; the double softmax runs on-chip.  Spatial PE term
  exp(-||x_j - x_8||/1000) rides extra stacked rows with Qs = S1*Q1 + S2*Q2.
  Temporal positional constant qp rides a ones-row; exp-overflow safety comes
  from a post-LRelu per-group constant shift.

  Batch rows inside the kernel are ordered (ts, b, ck) / (js, b, jq) so that
  all on-chip corner-turn DMAs are contiguous block copies; per-ts output DMAs
  unscramble to the reference layout.
"""
import sys
import numpy as np

for _p in ("/opt/trn_rl_repo", "/root/.axon_site/_ro/trn_rl_repo"):
    if _p not in sys.path:
        sys.path.insert(0, _p)

from contextlib import ExitStack  # noqa: E402

import concourse.bass as bass  # noqa: E402
import concourse.tile as tile  # noqa: E402
from concourse import bacc, mybir  # noqa: E402

B, C, T, J, F = 16, 128, 25, 25, 256
N = 25            # N == T == J
NN = N * N        # 625
NCORES = 8
BL = B // NCORES  # 2 batches per core
FP = mybir.dt.float32
BF = mybir.dt.bfloat16
AF = mybir.ActivationFunctionType
ALU = mybir.AluOpType

KS = 89           # spatial stack: 0:25 s1, 25:50 s2, 50:64 zero, 64:89 ec
KT = 57           # temporal stack: 0:25 t1, 25 ones, 26:32 zero, 32:57 t2

# n2-split for softmax-chain pipelining
N2SPLITS = [(0, 13), (13, 25)]

# Pin ALL activation functions to one table set (exp/ln/square/copy live
# together in natural_log_exp_and_others) so only one ACT_TABLE_LOAD happens.
_orig_get_tables = bacc.get_activation_tables


def _pinned_tables(arch):
    tabs = dict(_orig_get_tables(arch))
    assert "natural_log_exp_and_others" in tabs
    return {k: (v if k == "natural_log_exp_and_others" else set())
            for k, v in tabs.items()}


bacc.get_activation_tables = _pinned_tables

# ---------------------------------------------------------------- host math --

def _pair_indices():
    r1 = np.zeros(NN, np.int64)
    r2 = np.zeros(NN, np.int64)
    for m in range(NN):
        k1, k2 = 2 * m, 2 * m + 1
        r1[m] = (k1 // N) if k1 < NN else ((k1 - NN) % N)
        r2[m] = (k2 // N) if k2 < NN else ((k2 - NN) % N)
    return r1, r2


def _sinusoid_pos():
    pos = np.arange(200)[:, None].astype(np.float64)
    hid = np.arange(C)[None, :]
    angle = pos / np.power(10000.0, 2.0 * (hid // 2) / C)
    tab = angle.copy()
    tab[:, 0::2] = np.sin(angle[:, 0::2])
    tab[:, 1::2] = np.cos(angle[:, 1::2])
    return tab[:T] * 1000.0  # [T, C] float64


_R1, _R2 = _pair_indices()


def _host_consts(W_s, a_s, W_t, a_t):
    """Precompute tiny derived params in float64. ~0.3 MFLOP."""
    W_s = W_s.astype(np.float64)
    a_s = a_s.astype(np.float64)
    W_t = W_t.astype(np.float64)
    a_t = a_t.astype(np.float64)
    wa_s1 = W_s @ a_s[:F, 0]
    wa_s2 = W_s @ a_s[F:, 0]
    wa_t1 = W_t @ a_t[:F, 0]
    wa_t2 = W_t @ a_t[F:, 0]
    S1, S2 = wa_s1.sum(), wa_s2.sum()

    Q1 = np.zeros((N, NN), np.float64)
    Q2 = np.zeros((N, NN), np.float64)
    Q1[_R1, np.arange(NN)] = 1.0
    Q2[_R2, np.arange(NN)] = 1.0
    qs = S1 * Q1 + S2 * Q2

    pos = _sinusoid_pos()
    p1 = pos @ wa_t1
    p2 = pos @ wa_t2
    qp = p1[_R1] + p2[_R2]
    qLR = np.where(qp > 0, qp, 0.2 * qp)
    cq = qLR.reshape(N, N).max(axis=0)
    csh = cq[np.arange(NN) % N][None, :]        # [1, 625]

    wa4 = np.stack([wa_s1, wa_s2, wa_t1, wa_t2], axis=1)  # [128, 4]
    qstk_s = np.zeros((KS, NN), np.float64)
    qstk_s[0:N] = Q1
    qstk_s[N:2 * N] = Q2
    qstk_s[64:64 + N] = qs
    qstk_t = np.zeros((KT, NN), np.float64)
    qstk_t[0:N] = Q1
    qstk_t[N] = qp
    qstk_t[32:32 + N] = Q2
    return (wa4.astype(np.float32), qstk_s.astype(np.float32),
            qstk_t.astype(np.float32), csh.astype(np.float32))


# ------------------------------------------------------------- bass program --

def _build_program():
    nc = bacc.Bacc("TRN2", target_bir_lowering=False, debug=False)

    src_d = nc.dram_tensor("src_l", [BL, C, T, J], FP, kind="ExternalInput").ap()
    wa4_d = nc.dram_tensor("wa4", [C, 4], FP, kind="ExternalInput").ap()
    qss_d = nc.dram_tensor("qstk_s", [KS, NN], FP, kind="ExternalInput").ap()
    qst_d = nc.dram_tensor("qstk_t", [KT, NN], FP, kind="ExternalInput").ap()
    csh_d = nc.dram_tensor("csh", [1, NN], FP, kind="ExternalInput").ap()
    outs_d = nc.dram_tensor("out_s", [BL, T, N, N], FP, kind="ExternalOutput").ap()
    outt_d = nc.dram_tensor("out_t", [BL, T, N, N], FP, kind="ExternalOutput").ap()

    with tile.TileContext(nc) as tc, ExitStack() as ctx:
        consts = ctx.enter_context(tc.tile_pool(name="consts", bufs=1))
        data = ctx.enter_context(tc.tile_pool(name="data", bufs=1))
        pp = ctx.enter_context(tc.tile_pool(name="pp", bufs=1, space="PSUM"))

        # --- input first (X gets DMA priority) ---
        X = data.tile([C, BL * NN], FP)
        for b in range(BL):
            src_b = bass.AP(tensor=src_d.tensor, offset=src_d.offset + b * C * NN,
                            ap=[[NN, C], [1, NN]])
            nc.sync.dma_start(X[:, b * NN:(b + 1) * NN], src_b)
        FX = X[:].ap[0][0]

        wa4 = consts.tile([C, 4], FP)
        nc.sync.dma_start(wa4[:], wa4_d)

        # --- ACT table warm-up ---
        dummy = consts.tile([1, 2], FP)
        nc.vector.memset(dummy[:], 0.0)
        nc.scalar.activation(dummy[:], dummy[:], AF.Exp)

        ones_bf = consts.tile([C, 1], BF)
        nc.vector.memset(ones_bf[:], 1.0)

        # --- X_jt [128, (b, j, t)] for the temporal pass (ACT strided copy) ---
        X_jt = data.tile([C, BL * NN], FP)
        xin = bass.AP(tensor=X.tensor, offset=X.offset,
                      ap=[[FX, C], [NN, BL], [1, N], [N, N]])   # (c, b, j, t)
        nc.scalar.copy(X_jt[:], xin)

        # --- D2 = (X - ref)^2 in bf16, per b ---
        D = data.tile([C, BL * NN], FP)
        D2 = data.tile([C, BL * NN], BF)
        FD = D[:].ap[0][0]
        for b in range(BL):
            in0 = bass.AP(tensor=X.tensor, offset=X.offset + b * NN,
                          ap=[[FX, C], [N, N], [1, N]])
            ref = bass.AP(tensor=X.tensor, offset=X.offset + b * NN + 8,
                          ap=[[FX, C], [N, N], [0, N]])
            dout = bass.AP(tensor=D.tensor, offset=D.offset + b * NN,
                           ap=[[FD, C], [N, N], [1, N]])
            nc.gpsimd.tensor_tensor(dout, in0, ref, op=ALU.subtract)
            eng = nc.vector if b == 0 else nc.gpsimd
            eng.tensor_tensor(D2[:, b * NN:(b + 1) * NN],
                              D[:, b * NN:(b + 1) * NN],
                              D[:, b * NN:(b + 1) * NN], op=ALU.mult)

        # --- PE dot passes (chunked stationary) ---
        psum_E = pp.tile([114, 1024], FP)  # first: keeps 512-chunks bank-aligned
        psum_sd = pp.tile([125, 30], FP)   # col (b*5+ck)*3 + {0:s1,1:s2,2:d2}
        psum_td = pp.tile([125, 20], FP)   # col (b*5+jq)*2 + {t1,t2}
        for q in range(BL * 5):
            nc.tensor.matmul(psum_td[:, q * 2:q * 2 + 2],
                             X_jt[:, q * 125:(q + 1) * 125], wa4[:, 2:4],
                             start=True, stop=True)
        for q in range(BL * 5):
            nc.tensor.matmul(psum_sd[:, q * 3:q * 3 + 2],
                             X[:, q * 125:(q + 1) * 125], wa4[:, 0:2],
                             start=True, stop=True)
        for q in range(BL * 5):
            nc.tensor.matmul(psum_sd[:, q * 3 + 2:q * 3 + 3],
                             D2[:, q * 125:(q + 1) * 125], ones_bf[:],
                             start=True, stop=True)

        # --- PSUM -> SBUF with d-major column permute (lane-local) ---
        # TDp[p, d*10 + bjq] = psum_td[p, bjq*2 + d]
        TDp = data.tile([125, 20], FP)
        FTD = TDp[:].ap[0][0]
        td_out = bass.AP(tensor=TDp.tensor, offset=TDp.offset,
                         ap=[[FTD, 125], [1, 10], [10, 2]])      # (bjq, d)
        td_in = bass.AP(tensor=psum_td.tensor, offset=psum_td.offset,
                        ap=[[psum_td[:].ap[0][0], 125], [2, 10], [1, 2]])
        nc.vector.tensor_copy(td_out, td_in)
        # SDp[p, d*10 + bck] = psum_sd[p, bck*3 + d]
        SDp = data.tile([125, 30], FP)
        FSD = SDp[:].ap[0][0]
        sd_out = bass.AP(tensor=SDp.tensor, offset=SDp.offset,
                         ap=[[FSD, 125], [1, 10], [10, 3]])      # (bck, d)
        sd_in = bass.AP(tensor=psum_sd.tensor, offset=psum_sd.offset,
                        ap=[[psum_sd[:].ap[0][0], 125], [3, 10], [1, 3]])
        nc.scalar.copy(sd_out, sd_in)

        # --- stacked lhsT tiles; cols ordered (ts, b, ck) / (js, b, jq) ---
        SPK = data.tile([KS, 50], FP)
        nc.vector.memset(SPK[:], 0.0)
        TPK = data.tile([KT, 50], FP)
        nc.vector.memset(TPK[:], 0.0)
        onesrow = consts.tile([1, 50], FP)
        nc.vector.memset(onesrow[:], 1.0)
        nc.sync.dma_start(TPK[N:N + 1, :], onesrow[:])
        FSK = SPK[:].ap[0][0]
        FTK = TPK[:].ap[0][0]

        # temporal rearranges first (TD ready before SD's d2 part)
        for js in range(5):
            for d, rbase in ((0, 0), (1, 32)):
                src = bass.AP(tensor=TDp.tensor,
                              offset=TDp.offset + (js * 25) * FTD + d * 10,
                              ap=[[FTD, N], [1, 10]])
                dst = bass.AP(tensor=TPK.tensor,
                              offset=TPK.offset + rbase * FTK + js * 10,
                              ap=[[FTK, N], [1, 10]])
                nc.sync.dma_start(dst, src)
        for ts in range(5):
            for d, rbase in ((0, 0), (1, N), (2, 64)):
                src = bass.AP(tensor=SDp.tensor,
                              offset=SDp.offset + (ts * 25) * FSD + d * 10,
                              ap=[[FSD, N], [1, 10]])
                dst = bass.AP(tensor=SPK.tensor,
                              offset=SPK.offset + rbase * FSK + ts * 10,
                              ap=[[FSK, N], [1, 10]])
                nc.sync.dma_start(dst, src)

        # --- big consts (needed later; after X in DMA priority) ---
        qst = consts.tile([KT, NN], FP)
        nc.sync.dma_start(qst[:], qst_d)
        qss = consts.tile([KS, NN], FP)
        nc.sync.dma_start(qss[:], qss_d)
        CSHt = consts.tile([114, NN], FP)
        csh_b = bass.AP(tensor=csh_d.tensor, offset=csh_d.offset, ap=[[0, 50], [1, NN]])
        nc.gpsimd.dma_start(CSHt[64:114, :], csh_b)

        # --- EC = exp(-sqrt(d2s)/1000) via exp(0.5*ln) on SPK rows 64:89 ---
        eps_b = consts.tile([89, 1], FP)
        nc.vector.memset(eps_b[:], 1e-30)
        ecL = data.tile([89, 50], FP)
        nc.scalar.activation(ecL[64:89, :], SPK[64:89, 0:50], AF.Ln,
                             bias=eps_b[64:89])
        ecW = data.tile([89, 50], FP)
        nc.scalar.activation(ecW[64:89, :], ecL[64:89, :], AF.Exp, scale=0.5)
        nc.scalar.activation(SPK[64:89, 0:50], ecW[64:89, :], AF.Exp, scale=-0.001)

        # --- E matmuls (stacked-K): spatial rows 0:50, temporal 64:114 ---
        nc.vector.memset(psum_E[32:64, 0:NN], 0.0)  # junk rows 50:64
        chunks = [(0, 512), (512, NN)]
        for lo, hi in chunks:
            nc.tensor.matmul(psum_E[64:114, lo:hi], TPK[:, :], qst[:, lo:hi],
                             start=True, stop=True, tile_position=(0, 64))
        for lo, hi in chunks:
            nc.tensor.matmul(psum_E[0:50, lo:hi], SPK[:, :], qss[:, lo:hi],
                             start=True, stop=True)

        # --- softmax tail, pipelined over n2-halves ---
        t0 = data.tile([114, NN], FP)
        E2 = data.tile([114, NN], FP)
        g = data.tile([114, NN], FP)
        Z = data.tile([114, N], FP)
        Zr = data.tile([114, N], FP)
        att1 = data.tile([114, NN], FP)
        g2 = data.tile([114, NN], FP)
        Z2 = data.tile([114, N], FP)
        Z2r = data.tile([114, N], FP)
        outF = data.tile([114, NN], FP)
        FPE = psum_E[:].ap[0][0]

        def v3(t, lo, hi, npart=114, p0=0):
            """3D view [(p), (n2 in [lo:hi)), (n1 strided)] of a [*, 625] tile."""
            fs = t[:].ap[0][0]
            return bass.AP(tensor=t.tensor, offset=t.offset + p0 * fs + lo,
                           ap=[[fs, npart], [1, hi - lo], [N, N]])

        def v3p(lo, hi, npart=114, p0=0):
            return bass.AP(tensor=psum_E.tensor,
                           offset=psum_E.offset + p0 * FPE + lo,
                           ap=[[FPE, npart], [1, hi - lo], [N, N]])

        def v2d(t, lo, hi, npart=114, p0=0):
            fs = t[:].ap[0][0]
            return bass.AP(tensor=t.tensor, offset=t.offset + p0 * fs + lo,
                           ap=[[fs, npart], [1, hi - lo]])

        for lo, hi in N2SPLITS:
            # LRelu: E2 = max(E, 0.2E); temporal rows -csh
            nc.scalar.mul(v3(t0, lo, hi), v3p(lo, hi), 0.2)
            nc.vector.tensor_tensor(v3(E2, lo, hi), v3p(lo, hi), v3(t0, lo, hi),
                                    op=ALU.max)
            nc.vector.tensor_tensor(v3(E2, lo, hi, 50, 64), v3(E2, lo, hi, 50, 64),
                                    v3(CSHt, lo, hi, 50, 64), op=ALU.subtract)
            # softmax 1
            nc.scalar.activation(v3(g, lo, hi), v3(E2, lo, hi), AF.Exp)
            gr = bass.AP(tensor=g.tensor, offset=g.offset + lo,
                         ap=[[g[:].ap[0][0], 114], [1, hi - lo], [N, N]])
            nc.vector.tensor_reduce(v2d(Z, lo, hi), gr,
                                    axis=mybir.AxisListType.X, op=ALU.add)
            nc.vector.reciprocal(v2d(Zr, lo, hi), v2d(Z, lo, hi))
            zb = bass.AP(tensor=Zr.tensor, offset=Zr.offset + lo,
                         ap=[[Zr[:].ap[0][0], 114], [1, hi - lo], [0, N]])
            nc.vector.tensor_tensor(v3(att1, lo, hi), v3(g, lo, hi), zb,
                                    op=ALU.mult)
            # softmax 2
            nc.scalar.activation(v3(g2, lo, hi), v3(att1, lo, hi), AF.Exp)
            g2r = bass.AP(tensor=g2.tensor, offset=g2.offset + lo,
                          ap=[[g2[:].ap[0][0], 114], [1, hi - lo], [N, N]])
            nc.vector.tensor_reduce(v2d(Z2, lo, hi), g2r,
                                    axis=mybir.AxisListType.X, op=ALU.add)
            nc.vector.reciprocal(v2d(Z2r, lo, hi), v2d(Z2, lo, hi))
            z2b = bass.AP(tensor=Z2r.tensor, offset=Z2r.offset + lo,
                          ap=[[Z2r[:].ap[0][0], 114], [1, hi - lo], [0, N]])
            nc.vector.tensor_tensor(v3(outF, lo, hi), v3(g2, lo, hi), z2b,
                                    op=ALU.mult)

        # --- outputs: unscramble (ts,b,ck)-row order per ts / js ---
        FO = outF[:].ap[0][0]
        for ts in range(5):
            src = bass.AP(tensor=outF.tensor, offset=outF.offset + (ts * 10) * FO,
                          ap=[[FO, 10], [1, NN]])                # rows (b, ck)
            dst = bass.AP(tensor=outs_d.tensor, offset=outs_d.offset + ts * NN,
                          ap=[[25 * NN, BL], [5 * NN, 5], [1, NN]])  # (b, ck, m)
            nc.sync.dma_start(dst, src)
        for js in range(5):
            src = bass.AP(tensor=outF.tensor, offset=outF.offset + (64 + js * 10) * FO,
                          ap=[[FO, 10], [1, NN]])                # rows (b, jq)
            dst = bass.AP(tensor=outt_d.tensor, offset=outt_d.offset + js * NN,
                          ap=[[25 * NN, BL], [5 * NN, 5], [1, NN]])  # (b, jq, m)
            nc.sync.dma_start(dst, src)

    nc.compile()
    return nc


_PROGRAM = None


def _get_program():
    global _PROGRAM
    if _PROGRAM is None:
        _PROGRAM = _build_program()
    return _PROGRAM


# ------------------------------------------------------------------ kernel --

def kernel(src, W_s, a_s, W_t, a_t):
    from concourse.bass_utils import run_bass_kernel_spmd

    src = np.ascontiguousarray(np.asarray(src, dtype=np.float32))
    wa4, qstk_s, qstk_t, csh = _host_consts(np.asarray(W_s), np.asarray(a_s),
                                            np.asarray(W_t), np.asarray(a_t))
    nc = _get_program()
    in_maps = []
    for c in range(NCORES):
        in_maps.append({
            "src_l": src[c * BL:(c + 1) * BL],
            "wa4": wa4, "qstk_s": qstk_s, "qstk_t": qstk_t, "csh": csh,
        })
    res = run_bass_kernel_spmd(nc, in_maps, core_ids=list(range(NCORES)))
    out_s = np.concatenate([res.results[c]["out_s"] for c in range(NCORES)], axis=0)
    out_t = np.concatenate([res.results[c]["out_t"] for c in range(NCORES)], axis=0)
    return out_s, out_t


# revision 25
# speedup vs baseline: 1.2044x; 1.1397x over previous
"""Trainium2 Bass kernel for nn_DMS_STGAT (dual-branch GAT attention softmaxes).

Strategy (per core, data-parallel over batch B=16 -> 2 per core):
  The reference only uses h = x @ W through two dots s1 = h@a[:F], s2 = h@a[F:],
  so  e[bt, n1, n2] = LRelu(s1[r1[m]] + s2[r2[m]])  with fixed index maps r1/r2
  (the "scrambled pairing").  s1/s2 are 128-dim dots against host-precomputed
  Wa = W@a vectors, gathered via tiny host-precomputed 0/1 matrices using
  stacked-K PE matmuls; the double softmax runs on-chip.  Spatial PE term
  exp(-||x_j - x_8||/1000) rides extra stacked rows with Qs = S1*Q1 + S2*Q2.
  Temporal positional constant qp rides a ones-row; exp-overflow safety comes
  from a post-LRelu per-group constant shift.

  Batch rows inside the kernel are ordered (ts, b, ck) / (js, b, jq) so that
  all on-chip corner-turn DMAs are contiguous block copies; per-ts output DMAs
  unscramble to the reference layout.
"""
import sys
import numpy as np

for _p in ("/opt/trn_rl_repo", "/root/.axon_site/_ro/trn_rl_repo"):
    if _p not in sys.path:
        sys.path.insert(0, _p)

from contextlib import ExitStack  # noqa: E402

import concourse.bass as bass  # noqa: E402
import concourse.tile as tile  # noqa: E402
from concourse import bacc, mybir  # noqa: E402

B, C, T, J, F = 16, 128, 25, 25, 256
N = 25            # N == T == J
NN = N * N        # 625
NCORES = 8
BL = B // NCORES  # 2 batches per core
FP = mybir.dt.float32
BF = mybir.dt.bfloat16
AF = mybir.ActivationFunctionType
ALU = mybir.AluOpType

KS = 89           # spatial stack: 0:25 s1, 25:50 s2, 50:64 zero, 64:89 ec
KT = 57           # temporal stack: 0:25 t1, 25 ones, 26:32 zero, 32:57 t2

# n2-split for softmax-chain pipelining
N2SPLITS = [(0, 13), (13, 25)]

# Pin ALL activation functions to one table set (exp/ln/square/copy live
# together in natural_log_exp_and_others) so only one ACT_TABLE_LOAD happens.
_orig_get_tables = bacc.get_activation_tables


def _pinned_tables(arch):
    tabs = dict(_orig_get_tables(arch))
    assert "natural_log_exp_and_others" in tabs
    return {k: (v if k == "natural_log_exp_and_others" else set())
            for k, v in tabs.items()}


bacc.get_activation_tables = _pinned_tables

# ---------------------------------------------------------------- host math --

def _pair_indices():
    r1 = np.zeros(NN, np.int64)
    r2 = np.zeros(NN, np.int64)
    for m in range(NN):
        k1, k2 = 2 * m, 2 * m + 1
        r1[m] = (k1 // N) if k1 < NN else ((k1 - NN) % N)
        r2[m] = (k2 // N) if k2 < NN else ((k2 - NN) % N)
    return r1, r2


def _sinusoid_pos():
    pos = np.arange(200)[:, None].astype(np.float64)
    hid = np.arange(C)[None, :]
    angle = pos / np.power(10000.0, 2.0 * (hid // 2) / C)
    tab = angle.copy()
    tab[:, 0::2] = np.sin(angle[:, 0::2])
    tab[:, 1::2] = np.cos(angle[:, 1::2])
    return tab[:T] * 1000.0  # [T, C] float64


_R1, _R2 = _pair_indices()


def _host_consts(W_s, a_s, W_t, a_t):
    """Precompute tiny derived params in float64. ~0.3 MFLOP."""
    W_s = W_s.astype(np.float64)
    a_s = a_s.astype(np.float64)
    W_t = W_t.astype(np.float64)
    a_t = a_t.astype(np.float64)
    wa_s1 = W_s @ a_s[:F, 0]
    wa_s2 = W_s @ a_s[F:, 0]
    wa_t1 = W_t @ a_t[:F, 0]
    wa_t2 = W_t @ a_t[F:, 0]
    S1, S2 = wa_s1.sum(), wa_s2.sum()

    Q1 = np.zeros((N, NN), np.float64)
    Q2 = np.zeros((N, NN), np.float64)
    Q1[_R1, np.arange(NN)] = 1.0
    Q2[_R2, np.arange(NN)] = 1.0
    qs = S1 * Q1 + S2 * Q2

    pos = _sinusoid_pos()
    p1 = pos @ wa_t1
    p2 = pos @ wa_t2
    qp = p1[_R1] + p2[_R2]
    qLR = np.where(qp > 0, qp, 0.2 * qp)
    cq = qLR.reshape(N, N).max(axis=0)
    csh = cq[np.arange(NN) % N][None, :]        # [1, 625]

    wa4 = np.stack([wa_s1, wa_s2, wa_t1, wa_t2], axis=1)  # [128, 4]
    qstk_s = np.zeros((KS, NN), np.float64)
    qstk_s[0:N] = Q1
    qstk_s[N:2 * N] = Q2
    qstk_s[64:64 + N] = qs
    qstk_t = np.zeros((KT, NN), np.float64)
    qstk_t[0:N] = Q1
    qstk_t[N] = qp
    qstk_t[32:32 + N] = Q2
    return (wa4.astype(np.float32), qstk_s.astype(np.float32),
            qstk_t.astype(np.float32), csh.astype(np.float32))


# ------------------------------------------------------------- bass program --

def _build_program():
    nc = bacc.Bacc("TRN2", target_bir_lowering=False, debug=False)

    src_d = nc.dram_tensor("src_l", [BL, C, T, J], FP, kind="ExternalInput").ap()
    wa4_d = nc.dram_tensor("wa4", [C, 4], FP, kind="ExternalInput").ap()
    qss_d = nc.dram_tensor("qstk_s", [KS, NN], FP, kind="ExternalInput").ap()
    qst_d = nc.dram_tensor("qstk_t", [KT, NN], FP, kind="ExternalInput").ap()
    csh_d = nc.dram_tensor("csh", [1, NN], FP, kind="ExternalInput").ap()
    outs_d = nc.dram_tensor("out_s", [BL, T, N, N], FP, kind="ExternalOutput").ap()
    outt_d = nc.dram_tensor("out_t", [BL, T, N, N], FP, kind="ExternalOutput").ap()

    with tile.TileContext(nc) as tc, ExitStack() as ctx:
        consts = ctx.enter_context(tc.tile_pool(name="consts", bufs=1))
        data = ctx.enter_context(tc.tile_pool(name="data", bufs=1))
        pp = ctx.enter_context(tc.tile_pool(name="pp", bufs=1, space="PSUM"))

        # --- input first (X gets DMA priority) ---
        X = data.tile([C, BL * NN], FP)
        for b in range(BL):
            src_b = bass.AP(tensor=src_d.tensor, offset=src_d.offset + b * C * NN,
                            ap=[[NN, C], [1, NN]])
            nc.sync.dma_start(X[:, b * NN:(b + 1) * NN], src_b)
        FX = X[:].ap[0][0]

        wa4 = consts.tile([C, 4], FP)
        nc.sync.dma_start(wa4[:], wa4_d)

        # --- ACT table warm-up ---
        dummy = consts.tile([1, 2], FP)
        nc.vector.memset(dummy[:], 0.0)
        nc.scalar.activation(dummy[:], dummy[:], AF.Exp)

        ones_bf = consts.tile([C, 1], BF)
        nc.vector.memset(ones_bf[:], 1.0)

        # --- big consts on the ACT queue (issue early, transfer overlaps) ---
        qst = consts.tile([KT, NN], FP)
        nc.scalar.dma_start(qst[:], qst_d)
        qss = consts.tile([KS, NN], FP)
        nc.scalar.dma_start(qss[:], qss_d)

        # --- X_jt [128, (b, j, t)] for the temporal pass (ACT strided copy) ---
        X_jt = data.tile([C, BL * NN], FP)
        xin = bass.AP(tensor=X.tensor, offset=X.offset,
                      ap=[[FX, C], [NN, BL], [1, N], [N, N]])   # (c, b, j, t)
        nc.scalar.copy(X_jt[:], xin)

        # --- D2 = (X - ref)^2 in bf16, per b ---
        D = data.tile([C, BL * NN], FP)
        D2 = data.tile([C, BL * NN], BF)
        FD = D[:].ap[0][0]
        for b in range(BL):
            in0 = bass.AP(tensor=X.tensor, offset=X.offset + b * NN,
                          ap=[[FX, C], [N, N], [1, N]])
            ref = bass.AP(tensor=X.tensor, offset=X.offset + b * NN + 8,
                          ap=[[FX, C], [N, N], [0, N]])
            dout = bass.AP(tensor=D.tensor, offset=D.offset + b * NN,
                           ap=[[FD, C], [N, N], [1, N]])
            nc.gpsimd.tensor_tensor(dout, in0, ref, op=ALU.subtract)
            eng = nc.vector if b == 0 else nc.gpsimd
            eng.tensor_tensor(D2[:, b * NN:(b + 1) * NN],
                              D[:, b * NN:(b + 1) * NN],
                              D[:, b * NN:(b + 1) * NN], op=ALU.mult)

        # --- PE dot passes (chunked stationary) ---
        psum_E = pp.tile([114, 1024], FP)  # first: keeps 512-chunks bank-aligned
        psum_sd = pp.tile([125, 30], FP)   # col (b*5+ck)*3 + {0:s1,1:s2,2:d2}
        psum_td = pp.tile([125, 20], FP)   # col (b*5+jq)*2 + {t1,t2}
        for q in range(BL * 5):
            nc.tensor.matmul(psum_td[:, q * 2:q * 2 + 2],
                             X_jt[:, q * 125:(q + 1) * 125], wa4[:, 2:4],
                             start=True, stop=True)
        for q in range(BL * 5):
            nc.tensor.matmul(psum_sd[:, q * 3:q * 3 + 2],
                             X[:, q * 125:(q + 1) * 125], wa4[:, 0:2],
                             start=True, stop=True)
        for q in range(BL * 5):
            nc.tensor.matmul(psum_sd[:, q * 3 + 2:q * 3 + 3],
                             D2[:, q * 125:(q + 1) * 125], ones_bf[:],
                             start=True, stop=True)

        # --- PSUM -> SBUF with d-major column permute (lane-local) ---
        # TDp[p, d*10 + bjq] = psum_td[p, bjq*2 + d]
        TDp = data.tile([125, 20], FP)
        FTD = TDp[:].ap[0][0]
        td_out = bass.AP(tensor=TDp.tensor, offset=TDp.offset,
                         ap=[[FTD, 125], [1, 10], [10, 2]])      # (bjq, d)
        td_in = bass.AP(tensor=psum_td.tensor, offset=psum_td.offset,
                        ap=[[psum_td[:].ap[0][0], 125], [2, 10], [1, 2]])
        nc.vector.tensor_copy(td_out, td_in)
        # SDp[p, d*10 + bck] = psum_sd[p, bck*3 + d]
        SDp = data.tile([125, 30], FP)
        FSD = SDp[:].ap[0][0]
        sd_out = bass.AP(tensor=SDp.tensor, offset=SDp.offset,
                         ap=[[FSD, 125], [1, 10], [10, 3]])      # (bck, d)
        sd_in = bass.AP(tensor=psum_sd.tensor, offset=psum_sd.offset,
                        ap=[[psum_sd[:].ap[0][0], 125], [3, 10], [1, 3]])
        nc.vector.tensor_copy(sd_out, sd_in)

        # --- stacked lhsT tiles; cols ordered (ts, b, ck) / (js, b, jq) ---
        SPK = data.tile([KS, 50], FP)
        nc.vector.memset(SPK[:], 0.0)
        TPK = data.tile([KT, 50], FP)
        nc.vector.memset(TPK[:], 0.0)
        onesrow = consts.tile([1, 50], FP)
        nc.vector.memset(onesrow[:], 1.0)
        nc.gpsimd.dma_start(TPK[N:N + 1, :], onesrow[:])
        FSK = SPK[:].ap[0][0]
        FTK = TPK[:].ap[0][0]

        # temporal rearranges first (TD ready before SD's d2 part)
        for js in range(5):
            for d, rbase in ((0, 0), (1, 32)):
                src = bass.AP(tensor=TDp.tensor,
                              offset=TDp.offset + (js * 25) * FTD + d * 10,
                              ap=[[FTD, N], [1, 10]])
                dst = bass.AP(tensor=TPK.tensor,
                              offset=TPK.offset + rbase * FTK + js * 10,
                              ap=[[FTK, N], [1, 10]])
                nc.scalar.dma_start(dst, src)
        ns_rr = 0
        for d, rbase in ((2, 64), (0, 0), (1, N)):
            for ts in range(5):
                src = bass.AP(tensor=SDp.tensor,
                              offset=SDp.offset + (ts * 25) * FSD + d * 10,
                              ap=[[FSD, N], [1, 10]])
                dst = bass.AP(tensor=SPK.tensor,
                              offset=SPK.offset + rbase * FSK + ts * 10,
                              ap=[[FSK, N], [1, 10]])
                eng = nc.sync if ns_rr % 2 == 0 else nc.gpsimd
                ns_rr += 1
                eng.dma_start(dst, src)

        CSHt = consts.tile([114, NN], FP)
        csh_b = bass.AP(tensor=csh_d.tensor, offset=csh_d.offset, ap=[[0, 50], [1, NN]])
        nc.gpsimd.dma_start(CSHt[64:114, :], csh_b)

        # --- EC = exp(-sqrt(d2s)/1000) via exp(0.5*ln) on SPK rows 64:89 ---
        eps_b = consts.tile([89, 1], FP)
        nc.vector.memset(eps_b[:], 1e-30)
        ecL = data.tile([89, 50], FP)
        nc.scalar.activation(ecL[64:89, :], SPK[64:89, 0:50], AF.Ln,
                             bias=eps_b[64:89])
        ecW = data.tile([89, 50], FP)
        nc.scalar.activation(ecW[64:89, :], ecL[64:89, :], AF.Exp, scale=0.5)
        nc.scalar.activation(SPK[64:89, 0:50], ecW[64:89, :], AF.Exp, scale=-0.001)

        # --- E matmuls (stacked-K): spatial rows 0:50, temporal 64:114 ---
        nc.vector.memset(psum_E[32:64, 0:NN], 0.0)  # junk rows 50:64
        chunks = [(0, 512), (512, NN)]
        for lo, hi in chunks:
            nc.tensor.matmul(psum_E[64:114, lo:hi], TPK[:, :], qst[:, lo:hi],
                             start=True, stop=True, tile_position=(0, 64))
        for lo, hi in chunks:
            nc.tensor.matmul(psum_E[0:50, lo:hi], SPK[:, :], qss[:, lo:hi],
                             start=True, stop=True)

        # --- softmax tail: 3-engine pipeline over n2-halves ---
        t0 = data.tile([114, NN], FP)
        E2 = data.tile([114, NN], FP)
        g = data.tile([114, NN], FP)
        Z = data.tile([114, N], FP)
        Zr = data.tile([114, N], FP)
        att1 = data.tile([114, NN], FP)
        g2 = data.tile([114, NN], FP)
        Z2 = data.tile([114, N], FP)
        Z2r = data.tile([114, N], FP)
        outF = data.tile([114, NN], FP)


        def v3(t, lo, hi, npart=114, p0=0):
            """3D view [(p), (n2 in [lo:hi)), (n1 strided)] of a [*, 625] tile."""
            fs = t[:].ap[0][0]
            return bass.AP(tensor=t.tensor, offset=t.offset + p0 * fs + lo,
                           ap=[[fs, npart], [1, hi - lo], [N, N]])

        def v2d(t, lo, hi, npart=114, p0=0):
            fs = t[:].ap[0][0]
            return bass.AP(tensor=t.tensor, offset=t.offset + p0 * fs + lo,
                           ap=[[fs, npart], [1, hi - lo]])

        for lo, hi in N2SPLITS:
            # LRelu: E2 = max(E, 0.2E)  (ACT scales from psum, DVE maxes)
            pv = bass.AP(tensor=psum_E.tensor, offset=psum_E.offset + lo,
                         ap=[[psum_E[:].ap[0][0], 114], [1, hi - lo], [N, N]])
            nc.scalar.mul(v3(t0, lo, hi), pv, 0.2)
            nc.vector.tensor_tensor(v3(E2, lo, hi), pv, v3(t0, lo, hi),
                                    op=ALU.max)
            # temporal rows: subtract per-group shift
            nc.vector.tensor_tensor(v3(E2, lo, hi, 50, 64), v3(E2, lo, hi, 50, 64),
                                    v3(CSHt, lo, hi, 50, 64), op=ALU.subtract)
            # softmax 1
            nc.scalar.activation(v3(g, lo, hi), v3(E2, lo, hi), AF.Exp)
            gr = bass.AP(tensor=g.tensor, offset=g.offset + lo,
                         ap=[[g[:].ap[0][0], 114], [1, hi - lo], [N, N]])
            nc.vector.tensor_reduce(v2d(Z, lo, hi), gr,
                                    axis=mybir.AxisListType.X, op=ALU.add)
            nc.vector.reciprocal(v2d(Zr, lo, hi), v2d(Z, lo, hi))
            zb = bass.AP(tensor=Zr.tensor, offset=Zr.offset + lo,
                         ap=[[Zr[:].ap[0][0], 114], [1, hi - lo], [0, N]])
            nc.gpsimd.tensor_tensor(v3(att1, lo, hi), v3(g, lo, hi), zb,
                                    op=ALU.mult)
            # softmax 2
            nc.scalar.activation(v3(g2, lo, hi), v3(att1, lo, hi), AF.Exp)
            g2r = bass.AP(tensor=g2.tensor, offset=g2.offset + lo,
                          ap=[[g2[:].ap[0][0], 114], [1, hi - lo], [N, N]])
            nc.vector.tensor_reduce(v2d(Z2, lo, hi), g2r,
                                    axis=mybir.AxisListType.X, op=ALU.add)
            nc.vector.reciprocal(v2d(Z2r, lo, hi), v2d(Z2, lo, hi))
            z2b = bass.AP(tensor=Z2r.tensor, offset=Z2r.offset + lo,
                          ap=[[Z2r[:].ap[0][0], 114], [1, hi - lo], [0, N]])
            nc.vector.tensor_tensor(v3(outF, lo, hi), v3(g2, lo, hi), z2b,
                                    op=ALU.mult)

        # --- outputs: unscramble (ts,b,ck)-row order per ts / js ---
        FO = outF[:].ap[0][0]
        for ts in range(5):
            src = bass.AP(tensor=outF.tensor, offset=outF.offset + (ts * 10) * FO,
                          ap=[[FO, 10], [1, NN]])                # rows (b, ck)
            dst = bass.AP(tensor=outs_d.tensor, offset=outs_d.offset + ts * NN,
                          ap=[[25 * NN, BL], [5 * NN, 5], [1, NN]])  # (b, ck, m)
            nc.sync.dma_start(dst, src)
        for js in range(5):
            src = bass.AP(tensor=outF.tensor, offset=outF.offset + (64 + js * 10) * FO,
                          ap=[[FO, 10], [1, NN]])                # rows (b, jq)
            dst = bass.AP(tensor=outt_d.tensor, offset=outt_d.offset + js * NN,
                          ap=[[25 * NN, BL], [5 * NN, 5], [1, NN]])  # (b, jq, m)
            nc.scalar.dma_start(dst, src)

    nc.compile()
    return nc


_PROGRAM = None


def _get_program():
    global _PROGRAM
    if _PROGRAM is None:
        _PROGRAM = _build_program()
    return _PROGRAM


# ------------------------------------------------------------------ kernel --

def kernel(src, W_s, a_s, W_t, a_t):
    from concourse.bass_utils import run_bass_kernel_spmd

    src = np.ascontiguousarray(np.asarray(src, dtype=np.float32))
    wa4, qstk_s, qstk_t, csh = _host_consts(np.asarray(W_s), np.asarray(a_s),
                                            np.asarray(W_t), np.asarray(a_t))
    nc = _get_program()
    in_maps = []
    for c in range(NCORES):
        in_maps.append({
            "src_l": src[c * BL:(c + 1) * BL],
            "wa4": wa4, "qstk_s": qstk_s, "qstk_t": qstk_t, "csh": csh,
        })
    res = run_bass_kernel_spmd(nc, in_maps, core_ids=list(range(NCORES)))
    out_s = np.concatenate([res.results[c]["out_s"] for c in range(NCORES)], axis=0)
    out_t = np.concatenate([res.results[c]["out_t"] for c in range(NCORES)], axis=0)
    return out_s, out_t


# revision 28
# speedup vs baseline: 1.3985x; 1.1612x over previous
"""Trainium2 Bass kernel for nn_DMS_STGAT (dual-branch GAT attention softmaxes).

Strategy (per core, data-parallel over batch B=16 -> 2 per core):
  The reference only uses h = x @ W through two dots s1 = h@a[:F], s2 = h@a[F:],
  so  e[bt, n1, n2] = LRelu(s1[r1[m]] + s2[r2[m]])  with fixed index maps r1/r2
  (the "scrambled pairing").  s1/s2 are 128-dim dots against host-precomputed
  Wa = W@a vectors, gathered via tiny host-precomputed 0/1 matrices using
  stacked-K PE matmuls; the double softmax runs on-chip.  Spatial PE term
  exp(-||x_j - x_8||/1000) rides extra stacked rows with Qs = S1*Q1 + S2*Q2.
  Temporal positional constant qp rides a ones-row; exp-overflow safety comes
  from a post-LRelu per-group constant shift.

  Batch rows inside the kernel are ordered (ts, b, ck) / (js, b, jq) so that
  all on-chip corner-turn DMAs are contiguous block copies; per-ts output DMAs
  unscramble to the reference layout.
"""
import sys
import numpy as np

for _p in ("/opt/trn_rl_repo", "/root/.axon_site/_ro/trn_rl_repo"):
    if _p not in sys.path:
        sys.path.insert(0, _p)

from contextlib import ExitStack  # noqa: E402

import concourse.bass as bass  # noqa: E402
import concourse.tile as tile  # noqa: E402
from concourse import bacc, mybir  # noqa: E402

B, C, T, J, F = 16, 128, 25, 25, 256
N = 25            # N == T == J
NN = N * N        # 625
NCORES = 8
BL = B // NCORES  # 2 batches per core
FP = mybir.dt.float32
BF = mybir.dt.bfloat16
AF = mybir.ActivationFunctionType
ALU = mybir.AluOpType

KS = 89           # spatial stack: 0:25 s1, 25:50 s2, 50:64 zero, 64:89 ec
KT = 57           # temporal stack: 0:25 t1, 25 ones, 26:32 zero, 32:57 t2

# n2-split for softmax-chain pipelining
N2SPLITS = [(0, 13), (13, 25)]

# Pin ALL activation functions to one table set (exp/ln/square/copy live
# together in natural_log_exp_and_others) so only one ACT_TABLE_LOAD happens.
_orig_get_tables = bacc.get_activation_tables


def _pinned_tables(arch):
    tabs = dict(_orig_get_tables(arch))
    assert "natural_log_exp_and_others" in tabs
    return {k: (v if k == "natural_log_exp_and_others" else set())
            for k, v in tabs.items()}


bacc.get_activation_tables = _pinned_tables

# ---------------------------------------------------------------- host math --

def _pair_indices():
    r1 = np.zeros(NN, np.int64)
    r2 = np.zeros(NN, np.int64)
    for m in range(NN):
        k1, k2 = 2 * m, 2 * m + 1
        r1[m] = (k1 // N) if k1 < NN else ((k1 - NN) % N)
        r2[m] = (k2 // N) if k2 < NN else ((k2 - NN) % N)
    return r1, r2


def _sinusoid_pos():
    pos = np.arange(200)[:, None].astype(np.float64)
    hid = np.arange(C)[None, :]
    angle = pos / np.power(10000.0, 2.0 * (hid // 2) / C)
    tab = angle.copy()
    tab[:, 0::2] = np.sin(angle[:, 0::2])
    tab[:, 1::2] = np.cos(angle[:, 1::2])
    return tab[:T] * 1000.0  # [T, C] float64


_R1, _R2 = _pair_indices()


def _host_consts(W_s, a_s, W_t, a_t):
    """Precompute tiny derived params in float64. ~0.3 MFLOP."""
    W_s = W_s.astype(np.float64)
    a_s = a_s.astype(np.float64)
    W_t = W_t.astype(np.float64)
    a_t = a_t.astype(np.float64)
    wa_s1 = W_s @ a_s[:F, 0]
    wa_s2 = W_s @ a_s[F:, 0]
    wa_t1 = W_t @ a_t[:F, 0]
    wa_t2 = W_t @ a_t[F:, 0]
    S1, S2 = wa_s1.sum(), wa_s2.sum()

    Q1 = np.zeros((N, NN), np.float64)
    Q2 = np.zeros((N, NN), np.float64)
    Q1[_R1, np.arange(NN)] = 1.0
    Q2[_R2, np.arange(NN)] = 1.0
    qs = S1 * Q1 + S2 * Q2

    pos = _sinusoid_pos()
    p1 = pos @ wa_t1
    p2 = pos @ wa_t2
    qp = p1[_R1] + p2[_R2]
    qLR = np.where(qp > 0, qp, 0.2 * qp)
    cq = qLR.reshape(N, N).max(axis=0)
    csh = cq[np.arange(NN) % N][None, :]        # [1, 625]

    wa4 = np.stack([wa_s1, wa_s2, wa_t1, wa_t2], axis=1)  # [128, 4]
    # permute the m-axis to n2-major (m' = n2*25 + n1) so the softmax chain
    # and its n1-group reductions are contiguous on-chip
    mperm = (np.arange(NN) % N) * N + (np.arange(NN) // N)  # m' -> orig m
    qstk_s = np.zeros((KS, NN), np.float64)
    qstk_s[0:N] = Q1[:, mperm]
    qstk_s[N:2 * N] = Q2[:, mperm]
    qstk_s[64:64 + N] = qs[:, mperm]
    qstk_t = np.zeros((KT, NN), np.float64)
    qstk_t[0:N] = Q1[:, mperm]
    qstk_t[N] = qp[mperm]
    qstk_t[32:32 + N] = Q2[:, mperm]
    csh = cq[np.arange(NN) // N][None, :]       # n2-major
    return (wa4.astype(np.float32), qstk_s.astype(np.float32),
            qstk_t.astype(np.float32), csh.astype(np.float32))


# ------------------------------------------------------------- bass program --

def _build_program():
    nc = bacc.Bacc("TRN2", target_bir_lowering=False, debug=False)

    src_d = nc.dram_tensor("src_l", [BL, C, T, J], FP, kind="ExternalInput").ap()
    wa4_d = nc.dram_tensor("wa4", [C, 4], FP, kind="ExternalInput").ap()
    qss_d = nc.dram_tensor("qstk_s", [KS, NN], FP, kind="ExternalInput").ap()
    qst_d = nc.dram_tensor("qstk_t", [KT, NN], FP, kind="ExternalInput").ap()
    csh_d = nc.dram_tensor("csh", [1, NN], FP, kind="ExternalInput").ap()
    outs_d = nc.dram_tensor("out_s", [BL, T, N, N], FP, kind="ExternalOutput").ap()
    outt_d = nc.dram_tensor("out_t", [BL, T, N, N], FP, kind="ExternalOutput").ap()

    with tile.TileContext(nc) as tc, ExitStack() as ctx:
        consts = ctx.enter_context(tc.tile_pool(name="consts", bufs=1))
        data = ctx.enter_context(tc.tile_pool(name="data", bufs=1))
        pp = ctx.enter_context(tc.tile_pool(name="pp", bufs=1, space="PSUM"))

        # --- input first (X gets DMA priority) ---
        X = data.tile([C, BL * NN], FP)
        for b in range(BL):
            src_b = bass.AP(tensor=src_d.tensor, offset=src_d.offset + b * C * NN,
                            ap=[[NN, C], [1, NN]])
            nc.sync.dma_start(X[:, b * NN:(b + 1) * NN], src_b)
        FX = X[:].ap[0][0]

        wa4 = consts.tile([C, 4], FP)
        nc.sync.dma_start(wa4[:], wa4_d)

        # --- ACT table warm-up ---
        dummy = consts.tile([1, 2], FP)
        nc.vector.memset(dummy[:], 0.0)
        nc.scalar.activation(dummy[:], dummy[:], AF.Exp)

        ones_bf = consts.tile([C, 1], BF)
        nc.vector.memset(ones_bf[:], 1.0)

        # --- X_jt [128, (b, j, t)] for the temporal pass (ACT strided copy) ---
        X_jt = data.tile([C, BL * NN], FP)
        for b in range(BL):
            xin = bass.AP(tensor=X.tensor, offset=X.offset + b * NN,
                          ap=[[FX, C], [1, N], [N, N]])   # (c, j, t)
            nc.scalar.copy(X_jt[:, b * NN:(b + 1) * NN], xin)

        # --- big consts on the ACT queue (issued after X_jt; needed at E) ---
        qst = consts.tile([KT, NN], FP)
        nc.scalar.dma_start(qst[:], qst_d)
        qss = consts.tile([KS, NN], FP)
        nc.scalar.dma_start(qss[:], qss_d)

        # --- D2 = (X - ref)^2 in bf16, per b ---
        D = data.tile([C, BL * NN], FP)
        D2 = data.tile([C, BL * NN], BF)
        FD = D[:].ap[0][0]
        for b in range(BL):
            in0 = bass.AP(tensor=X.tensor, offset=X.offset + b * NN,
                          ap=[[FX, C], [N, N], [1, N]])
            ref = bass.AP(tensor=X.tensor, offset=X.offset + b * NN + 8,
                          ap=[[FX, C], [N, N], [0, N]])
            dout = bass.AP(tensor=D.tensor, offset=D.offset + b * NN,
                           ap=[[FD, C], [N, N], [1, N]])
            nc.gpsimd.tensor_tensor(dout, in0, ref, op=ALU.subtract)
            eng = nc.vector if b == 0 else nc.gpsimd
            eng.tensor_tensor(D2[:, b * NN:(b + 1) * NN],
                              D[:, b * NN:(b + 1) * NN],
                              D[:, b * NN:(b + 1) * NN], op=ALU.mult)

        # --- PE dot passes (chunked stationary) ---
        psum_E = pp.tile([114, 1024], FP)  # first: keeps 512-chunks bank-aligned
        psum_sd = pp.tile([125, 30], FP)   # col (b*5+ck)*3 + {0:s1,1:s2,2:d2}
        psum_td = pp.tile([125, 20], FP)   # col (b*5+jq)*2 + {t1,t2}
        for q in range(BL * 5):
            nc.tensor.matmul(psum_td[:, q * 2:q * 2 + 2],
                             X_jt[:, q * 125:(q + 1) * 125], wa4[:, 2:4],
                             start=True, stop=True)
        for q in range(BL * 5):
            nc.tensor.matmul(psum_sd[:, q * 3:q * 3 + 2],
                             X[:, q * 125:(q + 1) * 125], wa4[:, 0:2],
                             start=True, stop=True)
        for q in range(BL * 5):
            nc.tensor.matmul(psum_sd[:, q * 3 + 2:q * 3 + 3],
                             D2[:, q * 125:(q + 1) * 125], ones_bf[:],
                             start=True, stop=True)

        # --- PSUM -> SBUF with d-major column permute (lane-local) ---
        # TDp[p, d*10 + bjq] = psum_td[p, bjq*2 + d]
        TDp = data.tile([125, 20], FP)
        FTD = TDp[:].ap[0][0]
        td_out = bass.AP(tensor=TDp.tensor, offset=TDp.offset,
                         ap=[[FTD, 125], [1, 10], [10, 2]])      # (bjq, d)
        td_in = bass.AP(tensor=psum_td.tensor, offset=psum_td.offset,
                        ap=[[psum_td[:].ap[0][0], 125], [2, 10], [1, 2]])
        nc.vector.tensor_copy(td_out, td_in)
        # SDp[p, d*10 + bck] = psum_sd[p, bck*3 + d]
        SDp = data.tile([125, 30], FP)
        FSD = SDp[:].ap[0][0]
        sd_out = bass.AP(tensor=SDp.tensor, offset=SDp.offset,
                         ap=[[FSD, 125], [1, 10], [10, 3]])      # (bck, d)
        sd_in = bass.AP(tensor=psum_sd.tensor, offset=psum_sd.offset,
                        ap=[[psum_sd[:].ap[0][0], 125], [3, 10], [1, 3]])
        nc.vector.tensor_copy(sd_out, sd_in)

        # --- stacked lhsT tiles; cols ordered (ts, b, ck) / (js, b, jq) ---
        SPK = data.tile([KS, 50], FP)
        nc.vector.memset(SPK[:], 0.0)
        TPK = data.tile([KT, 50], FP)
        nc.vector.memset(TPK[:], 0.0)
        onesrow = consts.tile([1, 50], FP)
        nc.vector.memset(onesrow[:], 1.0)
        nc.gpsimd.dma_start(TPK[N:N + 1, :], onesrow[:])
        FSK = SPK[:].ap[0][0]
        FTK = TPK[:].ap[0][0]

        # temporal rearranges first (TD ready before SD's d2 part)
        for js in range(5):
            for d, rbase in ((0, 0), (1, 32)):
                src = bass.AP(tensor=TDp.tensor,
                              offset=TDp.offset + (js * 25) * FTD + d * 10,
                              ap=[[FTD, N], [1, 10]])
                dst = bass.AP(tensor=TPK.tensor,
                              offset=TPK.offset + rbase * FTK + js * 10,
                              ap=[[FTK, N], [1, 10]])
                eng = nc.sync if d == 0 else nc.gpsimd
                eng.dma_start(dst, src)
        for d, rbase, eng in ((2, 64, nc.scalar), (0, 0, nc.sync), (1, N, nc.gpsimd)):
            for ts in range(5):
                src = bass.AP(tensor=SDp.tensor,
                              offset=SDp.offset + (ts * 25) * FSD + d * 10,
                              ap=[[FSD, N], [1, 10]])
                dst = bass.AP(tensor=SPK.tensor,
                              offset=SPK.offset + rbase * FSK + ts * 10,
                              ap=[[FSK, N], [1, 10]])
                eng.dma_start(dst, src)

        CSHt = consts.tile([114, NN], FP)
        csh_b = bass.AP(tensor=csh_d.tensor, offset=csh_d.offset, ap=[[0, 50], [1, NN]])
        nc.gpsimd.dma_start(CSHt[64:114, :], csh_b)

        # --- EC = exp(-sqrt(d2s)/1000) via exp(0.5*ln) on SPK rows 64:89 ---
        eps_b = consts.tile([89, 1], FP)
        nc.vector.memset(eps_b[:], 1e-30)
        ecL = data.tile([89, 50], FP)
        nc.scalar.activation(ecL[64:89, :], SPK[64:89, 0:50], AF.Ln,
                             bias=eps_b[64:89])
        ecW = data.tile([89, 50], FP)
        nc.scalar.activation(ecW[64:89, :], ecL[64:89, :], AF.Exp, scale=0.5)
        nc.scalar.activation(SPK[64:89, 0:50], ecW[64:89, :], AF.Exp, scale=-0.001)

        # --- E matmuls (stacked-K): spatial rows 0:50, temporal 64:114 ---
        nc.vector.memset(psum_E[32:64, 0:NN], 0.0)  # junk rows 50:64
        chunks = [(0, 512), (512, NN)]
        for lo, hi in chunks:
            nc.tensor.matmul(psum_E[64:114, lo:hi], TPK[:, :], qst[:, lo:hi],
                             start=True, stop=True, tile_position=(0, 64))
        for lo, hi in chunks:
            nc.tensor.matmul(psum_E[0:50, lo:hi], SPK[:, :], qss[:, lo:hi],
                             start=True, stop=True)

        # --- softmax tail (m is n2-major: groups are contiguous 25-runs) ---
        t0 = data.tile([114, NN], FP)
        E2 = data.tile([114, NN], FP)
        g = data.tile([114, NN], FP)
        Z = data.tile([114, N], FP)
        Zr = data.tile([114, N], FP)
        att1 = data.tile([114, NN], FP)
        g2 = data.tile([114, NN], FP)
        Z2 = data.tile([114, N], FP)
        Z2r = data.tile([114, N], FP)
        outF = data.tile([114, NN], FP)

        def gview(t, lo, hi, npart=114, p0=0):
            """[(p), (n2 groups), (n1 contiguous)] view."""
            fs = t[:].ap[0][0]
            return bass.AP(tensor=t.tensor, offset=t.offset + p0 * fs + lo * N,
                           ap=[[fs, npart], [N, hi - lo], [1, N]])

        def bview(t, lo, hi, npart=114, p0=0):
            """broadcast [(p), (n2), (n1 step-0)] view of a [*, 25] tile."""
            fs = t[:].ap[0][0]
            return bass.AP(tensor=t.tensor, offset=t.offset + p0 * fs + lo,
                           ap=[[fs, npart], [1, hi - lo], [0, N]])

        for lo, hi in N2SPLITS:
            cl, ch = lo * N, hi * N
            # LRelu: E2 = max(E, 0.2E); temporal rows get -csh
            nc.scalar.mul(t0[:, cl:ch], psum_E[:, cl:ch], 0.2)
            nc.vector.tensor_tensor(E2[:, cl:ch], psum_E[:, cl:ch], t0[:, cl:ch],
                                    op=ALU.max)
            nc.gpsimd.tensor_tensor(E2[64:114, cl:ch], E2[64:114, cl:ch],
                                    CSHt[64:114, cl:ch], op=ALU.subtract)
            # softmax 1
            nc.scalar.activation(g[:, cl:ch], E2[:, cl:ch], AF.Exp)
            nc.vector.tensor_reduce(Z[:, lo:hi], gview(g, lo, hi),
                                    axis=mybir.AxisListType.X, op=ALU.add)
            nc.vector.reciprocal(Zr[:, lo:hi], Z[:, lo:hi])
            nc.gpsimd.tensor_tensor(gview(att1, lo, hi), gview(g, lo, hi),
                                    bview(Zr, lo, hi), op=ALU.mult)
            # softmax 2
            nc.scalar.activation(g2[:, cl:ch], att1[:, cl:ch], AF.Exp)
            nc.vector.tensor_reduce(Z2[:, lo:hi], gview(g2, lo, hi),
                                    axis=mybir.AxisListType.X, op=ALU.add)
            nc.vector.reciprocal(Z2r[:, lo:hi], Z2[:, lo:hi])
            # final scale, writing transposed back to n1-major for output
            oswap = bass.AP(tensor=outF.tensor, offset=outF.offset + lo,
                            ap=[[outF[:].ap[0][0], 114], [1, hi - lo], [N, N]])
            nc.gpsimd.tensor_tensor(oswap, gview(g2, lo, hi),
                                    bview(Z2r, lo, hi), op=ALU.mult)

        # --- outputs: unscramble (ts,b,ck)-row order per ts / js ---
        FO = outF[:].ap[0][0]
        for ts in range(5):
            src = bass.AP(tensor=outF.tensor, offset=outF.offset + (ts * 10) * FO,
                          ap=[[FO, 10], [1, NN]])                # rows (b, ck)
            dst = bass.AP(tensor=outs_d.tensor, offset=outs_d.offset + ts * NN,
                          ap=[[25 * NN, BL], [5 * NN, 5], [1, NN]])  # (b, ck, m)
            nc.sync.dma_start(dst, src)
        for js in range(5):
            src = bass.AP(tensor=outF.tensor, offset=outF.offset + (64 + js * 10) * FO,
                          ap=[[FO, 10], [1, NN]])                # rows (b, jq)
            dst = bass.AP(tensor=outt_d.tensor, offset=outt_d.offset + js * NN,
                          ap=[[25 * NN, BL], [5 * NN, 5], [1, NN]])  # (b, jq, m)
            nc.scalar.dma_start(dst, src)

    nc.compile()
    return nc


_PROGRAM = None


def _get_program():
    global _PROGRAM
    if _PROGRAM is None:
        _PROGRAM = _build_program()
    return _PROGRAM


# ------------------------------------------------------------------ kernel --

def kernel(src, W_s, a_s, W_t, a_t):
    from concourse.bass_utils import run_bass_kernel_spmd

    src = np.ascontiguousarray(np.asarray(src, dtype=np.float32))
    wa4, qstk_s, qstk_t, csh = _host_consts(np.asarray(W_s), np.asarray(a_s),
                                            np.asarray(W_t), np.asarray(a_t))
    nc = _get_program()
    in_maps = []
    for c in range(NCORES):
        in_maps.append({
            "src_l": src[c * BL:(c + 1) * BL],
            "wa4": wa4, "qstk_s": qstk_s, "qstk_t": qstk_t, "csh": csh,
        })
    res = run_bass_kernel_spmd(nc, in_maps, core_ids=list(range(NCORES)))
    out_s = np.concatenate([res.results[c]["out_s"] for c in range(NCORES)], axis=0)
    out_t = np.concatenate([res.results[c]["out_t"] for c in range(NCORES)], axis=0)
    return out_s, out_t
